# revision 46
# baseline (speedup 1.0000x reference)
"""AlloLayer forward on 8 TRN2 NeuronCores.

Math: reference computes
    lp   = log_softmax(hs, -1)                      # [B,T,C]
    ex   = exp(lp[..., phone_lab] + alloW)          # [B,T,A]
    sq   = scatter_add(ex, phoneme_lab)             # [B,T,P]
    red  = (sq.sum(-1) - 1) / P
    out  = log(sq - red)

The gather+exp+scatter collapses into a matmul: with
    M[c,p] = sum_{a: phone_lab[a]==c, phoneme_lab[a]==p} exp(alloW[a])
we have sq = softmax(hs) @ M.  Fold the redistribution term into the matrix,
    M'[c,p] = M[c,p] + (1 - sum_q M[c,q]) / P,      maug = [M' | ones]
so with U = exp(X) @ maug (PE, fp8/bf16 operands, f32 PSUM accumulate):
    numer = U[:,0:256]   s = U[:,256]   out = Ln(numer) - ln(s)
i.e. ONE batched ACT Ln per 512-row slice (covers numer and s of all four
128-row blocks via a 512-float-strided PSUM read), then a per-block DVE
tensor_scalar subtract that also downcasts to the bf16 output.

Data-parallel over B*T rows: 16384 rows -> 2048 rows per core, no cross-core
communication; output gathered on host.  Per-core shard is shipped fp8-e4m3,
pre-transposed AND pre-packed per 512-row slice ([128, 4*rs] with the
contraction dim on SBUF partitions, k-major per partition) so each slice is
ONE fully-contiguous input DMA and exp output feeds the matmul as lhsT with
no on-chip transpose.  Outputs are written in the device-natural packed
layout and inverse-permuted on the host.  maug is precomputed on host.

Engine budget per core (~best understood): ACT is the wall (exp 8192 +
Ln ~4112 elems/partition ~= 12us busy); PE ~7us (fp8 runs at bf16 rate
without DoubleRow), warmed up during the DMA fill by dummy matmuls so its
clock ramp (0.65/1.2/2.4 GHz p-states) completes before real work; DMA
~7.5us span (fp8 in + bf16 out = 2MB/core).  Ln work for slice i is
emitted one slice behind exp (ln_lag) so the in-order ACT stream never
stalls on the PE.
"""

import numpy as np
import ml_dtypes

import concourse.bass as bass
import concourse.tile as tile
from concourse import bacc, mybir
from concourse import bass_utils

F32 = mybir.dt.float32
BF16 = mybir.dt.bfloat16

N_CORES = 8
B, T, C, A, P = 16, 1024, 512, 4096, 256
ROWS = B * T                      # 16384
R_PER_CORE = ROWS // N_CORES      # 2048
NAUG = P + 3                      # 259: [M | (M@1)/P | ones | ones/P]
BLK = 128
KCH = C // BLK                    # 4 contraction chunks

# single source of truth for the shipped configuration (used by run() and
# by test.py's timing section).  BEST_BUILDER picks the graph builder;
# BEST_CFG are its kwargs.
BEST_BUILDER = "v2"
BEST_CFG = dict(x_fp8=True, packed_io=True, ln_lag=1, pe_warm=14)


def _pin_act_table(arch):
    """Make natural_log_exp_and_others the only table-set advertising Exp/Ln.

    The compiled NEFF then keeps one resident ACT table set for the whole
    kernel instead of reloading (~1.3us each) on every Exp<->Ln alternation.
    Temporarily mutates the functools-cached dict (set indices unchanged; the
    real HW set genuinely contains both functions, so execution is
    unaffected); returns a restore() closure to undo it after compile.
    """
    from concourse import hw_specs

    tabs = hw_specs.get_activation_tables(arch)
    both = "natural_log_exp_and_others"
    assert both in tabs
    af = mybir.ActivationFunctionType
    assert af.Exp in tabs[both] and af.Ln in tabs[both]
    removed = []
    for name, fns in tabs.items():
        if name != both:
            for f in (af.Exp, af.Ln):
                if f in fns:
                    fns.discard(f)
                    removed.append((fns, f))

    def restore():
        for fns, f in removed:
            fns.add(f)

    return restore


def build_graph_t(x_bufs=3, e_bufs=3, o_bufs=3, u_bufs=6, out_batch=1,
                  rs=512, in_split=2, exp_split=1, out_eng="sync",
                  in_eng="sync", out_blocks=64, exp_mode="k", maug_bf16=False,
                  ln_batch=1, x_bf16=False, out_bf16=False, rsched=None,
                  bench_iters=0, skip_mm=False, skip_dve=False,
                  dma_only=False, compute_only=False):
    """Kernel taking the per-core x shard PRE-TRANSPOSED on the host:
    x_t[C, R_PER_CORE].  No on-chip transposes: DMA loads [128c, r] tiles
    directly, exp runs on big tiles, PE does only the matmuls.

    dma_only: skip all compute; out tiles only memset and streamed out.
    compute_only: skip both in- and out-DMA (xs memset instead).
    """
    from contextlib import ExitStack, nullcontext

    nc = bacc.Bacc("TRN2", target_bir_lowering=False, debug=False, num_devices=1)
    _restore_tabs = _pin_act_table(nc.m.arch)
    x_dt = BF16 if x_bf16 else F32
    out_dt = BF16 if out_bf16 else F32
    x_ap = nc.dram_tensor("x", [C, R_PER_CORE], x_dt, kind="ExternalInput").ap()
    maug_dt = BF16 if maug_bf16 else F32
    maug_ap = nc.dram_tensor("maug", [KCH, BLK, NAUG], maug_dt, kind="ExternalInput").ap()
    out_ap = nc.dram_tensor("out", [R_PER_CORE, P], out_dt, kind="ExternalOutput").ap()

    if rsched is None:
        rsched = [rs] * (R_PER_CORE // rs)
    assert sum(rsched) == R_PER_CORE

    with tile.TileContext(nc) as tc, ExitStack() as ctx:
        const_pool = ctx.enter_context(tc.tile_pool(name="const", bufs=1))
        x_pool = ctx.enter_context(tc.tile_pool(name="xin", bufs=x_bufs))
        e_pool = ctx.enter_context(tc.tile_pool(name="e", bufs=e_bufs))
        o_pool = ctx.enter_context(tc.tile_pool(name="o", bufs=o_bufs))
        s_pool = ctx.enter_context(tc.tile_pool(name="small", bufs=4))
        v_pool = ctx.enter_context(tc.tile_pool(name="v", bufs=3))
        u_pool = ctx.enter_context(tc.tile_pool(name="u", bufs=u_bufs, space="PSUM"))

        maug_sb = const_pool.tile([BLK, KCH * NAUG], BF16)
        nc.gpsimd.dma_start(
            maug_sb[:].rearrange("p (k n) -> p k n", n=NAUG),
            maug_ap[:, :, :].rearrange("k p n -> p k n"),
        )

        loop_cm = (
            tc.For_i(0, abs(bench_iters), 1) if bench_iters else nullcontext()
        )
        ctx.enter_context(loop_cm)

        x_t3 = x_ap.rearrange("(k p) r -> k p r", p=BLK)   # [KCH, 128, R]
        kper = KCH // in_split                              # c-chunks per in-DMA
        RSMAX = max(rsched)
        r0 = 0
        for rs in rsched:
            BPRS = rs // BLK
            # x slice [128, KCH*rs]: c-chunk k occupies cols [k*rs, (k+1)*rs)
            xs = x_pool.tile([BLK, KCH * RSMAX], x_dt, tag="xs")
            if compute_only:
                nc.vector.memset(xs[:, 0:8], 0.0)
            else:
                for d in range(in_split):
                    deng = nc.sync if (in_eng == "sync" or d % 2 == 0) else nc.scalar
                    deng.dma_start(
                        xs[:, d * kper * rs:(d + 1) * kper * rs].rearrange(
                            "p (k r) -> p k r", r=rs
                        ),
                        x_t3[d * kper:(d + 1) * kper, :, r0:r0 + rs].rearrange(
                            "k p r -> p k r"
                        ),
                    )
            outs = o_pool.tile([BLK, (RSMAX // BLK) * P], out_dt, tag="outs")
            if dma_only:
                nc.vector.memset(outs[:, 0:8], 0.0)
            else:
                e = e_pool.tile([BLK, KCH * RSMAX], BF16, tag="e")
                if exp_mode == "block":
                    # one exp per row-block spanning all 4 c-chunks (strided
                    # AP): each block's matmuls wait on ONE exp, not all
                    x3 = xs[:, 0:KCH * rs].rearrange("p (k r) -> p k r", r=rs)
                    e3 = e[:, 0:KCH * rs].rearrange("p (k r) -> p k r", r=rs)
                    for b in range(rs // BLK):
                        nc.scalar.activation(
                            e3[:, :, b * BLK:(b + 1) * BLK],
                            x3[:, :, b * BLK:(b + 1) * BLK],
                            mybir.ActivationFunctionType.Exp,
                        )
                else:
                    estep = KCH * rs // exp_split
                    for s in range(exp_split):
                        nc.scalar.activation(
                            e[:, s * estep:(s + 1) * estep],
                            xs[:, s * estep:(s + 1) * estep],
                            mybir.ActivationFunctionType.Exp,
                        )
                if skip_mm:
                    nc.vector.memset(outs[:, 0:8], 0.0)
            for b in range(BPRS if not (skip_mm or dma_only) else 0):
                u = u_pool.tile([BLK, NAUG], F32, tag="u")
                for k in range(KCH):
                    nc.tensor.matmul(
                        u[:],
                        e[:, k * rs + b * BLK:k * rs + (b + 1) * BLK],
                        maug_sb[:, k * NAUG:(k + 1) * NAUG],
                        start=(k == 0),
                        stop=(k == KCH - 1),
                    )
                if skip_dve:
                    nc.scalar.activation(
                        outs[:, b * P:(b + 1) * P],
                        u[:, 0:P],
                        mybir.ActivationFunctionType.Ln,
                        bias=0.0,
                        scale=1.0,
                    )
                    continue
                inv_s = s_pool.tile([BLK, 1], F32, tag="inv")
                nc.vector.reciprocal(inv_s[:], u[:, P + 1:P + 2])
                bias_t = s_pool.tile([BLK, 1], F32, tag="bias")
                if ln_batch > 1:
                    # normalize on DVE (per-partition scalars), then one Ln
                    # per ln_batch blocks — fewer serial ACT instructions
                    if b % ln_batch == 0:
                        v = v_pool.tile([BLK, ln_batch * P], F32, tag="v")
                    # bias2 = s/P - w/P = (s - w)/P
                    nc.vector.tensor_scalar(
                        bias_t[:],
                        u[:, P + 2:P + 3],
                        u[:, P:P + 1],
                        None,
                        op0=mybir.AluOpType.subtract,
                    )
                    # V = (U + bias2) * inv_s
                    nc.vector.tensor_scalar(
                        v[:, (b % ln_batch) * P:(b % ln_batch + 1) * P],
                        u[:, 0:P],
                        bias_t[:],
                        inv_s[:],
                        op0=mybir.AluOpType.add,
                        op1=mybir.AluOpType.mult,
                    )
                    if (b + 1) % ln_batch == 0 or b == BPRS - 1:
                        g0 = (b // ln_batch) * ln_batch
                        ng = b - g0 + 1
                        nc.scalar.activation(
                            outs[:, g0 * P:(g0 + ng) * P],
                            v[:, 0:ng * P],
                            mybir.ActivationFunctionType.Ln,
                        )
                else:
                    # bias = (s/P - w/P) * (1/s) = (s - w)/(P*s)
                    nc.vector.scalar_tensor_tensor(
                        bias_t[:],
                        u[:, P + 2:P + 3],
                        u[:, P:P + 1],
                        inv_s[:],
                        op0=mybir.AluOpType.subtract,
                        op1=mybir.AluOpType.mult,
                    )
                    nc.scalar.activation(
                        outs[:, b * P:(b + 1) * P],
                        u[:, 0:P],
                        mybir.ActivationFunctionType.Ln,
                        bias=bias_t[:],
                        scale=inv_s[:],
                    )
            if not compute_only:
                for b0 in range(0, BPRS, out_blocks):
                    nb = min(out_blocks, BPRS - b0)
                    getattr(nc, out_eng).dma_start(
                        out_ap[r0 + b0 * BLK:r0 + (b0 + nb) * BLK, :].rearrange(
                            "(b p) c -> p b c", p=BLK
                        ),
                        outs[:, b0 * P:(b0 + nb) * P].rearrange(
                            "p (b c) -> p b c", c=P
                        ),
                    )
            r0 += rs
    try:
        nc.compile()
    finally:
        _restore_tabs()
    return nc


def build_graph_v2(rsched=None, rs=512, e_bufs=3, o_bufs=3, lnu_bufs=3,
                   out_blocks=0, out_eng="sync", in_split=1, x_bf16=True,
                   out_bf16=True, x_fp8=False, ln_lag=1, pe_warm=0,
                   packed_io=False, fill_split=1, sub_eng="vector",
                   exp_rsplit=1, tail_split=False, bench_iters=0,
                   dma_only=False, compute_only=False, stage="all"):
    """Restructured kernel:

    - maug folds the redistribution term: M'[c,p] = M[c,p] + (1-mrow[c])/P,
      plus a ones column -> U = e @ [M' | 1] gives numer and s directly and
      out = Ln(numer) - ln(s); the Ln needs no per-block scale/bias, so it
      batches across a whole slice.
    - per-slice PSUM tile [128, BPRS*512] f32 (one bank per 128-row block,
      [numer | s] in cols [b*512, b*512+257)); ONE strided ACT Ln per slice
      covers numer and s of every block.
    - ln(s) subtract + bf16 downcast on DVE, per block.
    - all in-DMAs emitted first (no head-of-line blocking on SP), one DMA
      per slice; outputs bf16.
    - ln_lag: slices of Ln/subtract/out work are emitted that many slices
      behind the exp/matmul front, so the in-order ACT stream never stalls
      waiting on a cold PE.
    """
    from contextlib import ExitStack, nullcontext

    nc = bacc.Bacc("TRN2", target_bir_lowering=False, debug=False, num_devices=1)
    _restore_tabs = _pin_act_table(nc.m.arch)
    x_dt = mybir.dt.float8e4 if x_fp8 else (BF16 if x_bf16 else F32)
    out_dt = BF16 if out_bf16 else F32
    NA = P + 1                 # 257: [M' | ones]
    UST = 512                  # psum floats per block (bank-aligned)
    if rsched is None:
        rsched = [rs] * (R_PER_CORE // rs)
    assert sum(rsched) == R_PER_CORE
    if packed_io:
        # host pre-permutes x so each slice is one fully contiguous
        # [128, KCH*rs] block (per-partition DMA runs of KCH*rs elements
        # instead of rs), and the output is written in the device-natural
        # [slice][p, blk*c] order (host inverse-permutes after gather);
        # uniform slices only
        assert all(r == rsched[0] for r in rsched)
        x_ap = nc.dram_tensor("x", [len(rsched), BLK, KCH * rsched[0]], x_dt,
                              kind="ExternalInput").ap()
        out_ap = nc.dram_tensor("out", [len(rsched), BLK, (rsched[0] // BLK) * P],
                                out_dt, kind="ExternalOutput").ap()
    else:
        x_ap = nc.dram_tensor("x", [C, R_PER_CORE], x_dt, kind="ExternalInput").ap()
        out_ap = nc.dram_tensor("out", [R_PER_CORE, P], out_dt,
                                kind="ExternalOutput").ap()
    maug_ap = nc.dram_tensor("maug", [KCH, BLK, NA], F32, kind="ExternalInput").ap()
    NS = len(rsched)
    # live slice-level PSUM tiles (ring slots sized by the largest slice)
    # must fit in the 8 PSUM banks
    u_bufs = max(2, ln_lag + 1)
    assert u_bufs * max(rsched) <= 1024

    with tile.TileContext(nc) as tc, ExitStack() as ctx:
        const_pool = ctx.enter_context(tc.tile_pool(name="const", bufs=1))
        x_pool = ctx.enter_context(tc.tile_pool(name="xin", bufs=NS))
        e_pool = ctx.enter_context(tc.tile_pool(name="e", bufs=e_bufs))
        lnu_pool = ctx.enter_context(tc.tile_pool(name="lnu", bufs=lnu_bufs))
        o_pool = ctx.enter_context(tc.tile_pool(name="o", bufs=o_bufs))
        s_pool = ctx.enter_context(tc.tile_pool(name="small", bufs=4))
        u_pool = ctx.enter_context(tc.tile_pool(name="u", bufs=u_bufs, space="PSUM"))

        maug_sb = const_pool.tile([BLK, KCH * NA], BF16)
        nc.gpsimd.dma_start(
            maug_sb[:].rearrange("p (k n) -> p k n", n=NA),
            maug_ap[:, :, :].rearrange("k p n -> p k n"),
        )
        zwarm = None
        if pe_warm:
            zwarm = const_pool.tile([BLK, 2 * BLK], BF16)
            nc.vector.memset(zwarm[:], 0.0)

        loop_cm = (
            tc.For_i(0, abs(bench_iters), 1) if bench_iters else nullcontext()
        )
        ctx.enter_context(loop_cm)

        if pe_warm and not dma_only:
            # dummy matmuls keep the PE p-state ramp going during the DMA
            # fill window so the first real matmuls run at full clock
            # (borrows a u-pool slot; freed before slice 1 needs it)
            wu = u_pool.tile([BLK, (max(rsched) // BLK) * UST], F32, tag="u")
            for i in range(pe_warm):
                nc.tensor.matmul(
                    wu[:, 0:2 * BLK], zwarm[:, 0:BLK], zwarm[:, 0:2 * BLK],
                    start=True, stop=True,
                )

        x_t3 = None if packed_io else x_ap.rearrange("(k p) r -> k p r", p=BLK)
        RSMAX = max(rsched)
        kper = KCH // in_split

        # phase A: all input DMAs, one (or in_split) per slice, on SP HWDGE
        xs_tiles = []
        r0 = 0
        for si, rs_ in enumerate(rsched):
            xs = x_pool.tile([BLK, KCH * RSMAX], x_dt, tag="xs")
            if compute_only:
                nc.vector.memset(xs[:, 0:8], 0.0)
            elif packed_io:
                isp = fill_split if (si == 0 and fill_split > 1) else in_split
                step = KCH * rs_ // isp
                for d in range(isp):
                    nc.sync.dma_start(
                        xs[:, d * step:(d + 1) * step],
                        x_ap[si, :, d * step:(d + 1) * step],
                    )
            else:
                for d in range(in_split):
                    nc.sync.dma_start(
                        xs[:, d * kper * rs_:(d + 1) * kper * rs_].rearrange(
                            "p (k r) -> p k r", r=rs_
                        ),
                        x_t3[d * kper:(d + 1) * kper, :, r0:r0 + rs_].rearrange(
                            "k p r -> p k r"
                        ),
                    )
            xs_tiles.append(xs)
            r0 += rs_

        # phase B: per slice: exp -> matmuls; Ln/subtract/out trail ln_lag
        # slices behind so the in-order ACT stream never waits on the PE.
        NAW = NA  # columns per block read by the batched Ln ([numer | s])

        def emit_out_dma(BPRS, si_, r0_, outs):
            ob = out_blocks if out_blocks else BPRS
            for b0 in range(0, BPRS, ob):
                nb = min(ob, BPRS - b0)
                if packed_io:
                    getattr(nc, out_eng).dma_start(
                        out_ap[si_, :, b0 * P:(b0 + nb) * P],
                        outs[:, b0 * P:(b0 + nb) * P],
                    )
                else:
                    getattr(nc, out_eng).dma_start(
                        out_ap[r0_ + b0 * BLK:r0_ + (b0 + nb) * BLK, :].rearrange(
                            "(b p) c -> p b c", p=BLK
                        ),
                        outs[:, b0 * P:(b0 + nb) * P].rearrange(
                            "p (b c) -> p b c", c=P
                        ),
                    )

        def emit_ln(rs_, u, r0_, si_, skip_dma=False, per_block=False):
            if stage in ("exp", "mm", "mm1"):
                return
            BPRS = rs_ // BLK
            outs = o_pool.tile([BLK, (RSMAX // BLK) * P], out_dt, tag="outs")
            u3 = u[:, 0:BPRS * UST].rearrange("p (b x) -> p b x", x=UST)
            lnu = lnu_pool.tile([BLK, (RSMAX // BLK) * NAW], F32, tag="lnu")
            lnu3 = lnu[:, 0:BPRS * NAW].rearrange("p (b x) -> p b x", x=NAW)
            if not per_block:
                # one ACT op: ln of numer AND s for every block of the slice
                nc.scalar.activation(
                    lnu3[:, :, :], u3[:, :, 0:NAW],
                    mybir.ActivationFunctionType.Ln,
                )
            if stage == "ln" and not per_block:
                return
            for b in range(BPRS):
                if per_block:
                    # drain mode: Ln -> subtract -> out-DMA pipelined per
                    # block so the kernel tail is short
                    nc.scalar.activation(
                        lnu3[:, b:b + 1, :], u3[:, b:b + 1, 0:NAW],
                        mybir.ActivationFunctionType.Ln,
                    )
                seng = nc.vector if (sub_eng == "vector" or b % 2 == 0) else nc.gpsimd
                seng.tensor_scalar(
                    outs[:, b * P:(b + 1) * P],
                    lnu[:, b * NAW:b * NAW + P],
                    lnu[:, b * NAW + P:b * NAW + P + 1],
                    None,
                    op0=mybir.AluOpType.subtract,
                )
                if per_block and not skip_dma:
                    if packed_io:
                        getattr(nc, out_eng).dma_start(
                            out_ap[si_, :, b * P:(b + 1) * P],
                            outs[:, b * P:(b + 1) * P],
                        )
                    else:
                        getattr(nc, out_eng).dma_start(
                            out_ap[r0_ + b * BLK:r0_ + (b + 1) * BLK, :].rearrange(
                                "(o p) c -> p o c", p=BLK
                            ),
                            outs[:, b * P:(b + 1) * P].rearrange(
                                "p (o c) -> p o c", c=P
                            ),
                        )
            if skip_dma or per_block:
                return
            emit_out_dma(BPRS, si_, r0_, outs)

        if dma_only:
            r0 = 0
            for si, rs_ in enumerate(rsched):
                BPRS = rs_ // BLK
                outs = o_pool.tile([BLK, (RSMAX // BLK) * P], out_dt, tag="outs")
                nc.vector.memset(outs[:, 0:8], 0.0)
                emit_out_dma(BPRS, si, r0, outs)
                r0 += rs_
        else:
            pending = []
            r0 = 0
            for si, rs_ in enumerate(rsched):
                BPRS = rs_ // BLK
                xs = xs_tiles[si]
                e = e_pool.tile([BLK, KCH * RSMAX], BF16, tag="e")
                esp = fill_split if (si == 0 and fill_split > 1 and packed_io
                                     and not compute_only) else 1
                if exp_rsplit > 1 and esp == 1:
                    # split the slice's exp by ROW halves (strided 3D APs):
                    # the first blocks' matmuls start after half the exp
                    x3 = xs[:, 0:KCH * rs_].rearrange("p (k r) -> p k r", r=rs_)
                    e3 = e[:, 0:KCH * rs_].rearrange("p (k r) -> p k r", r=rs_)
                    rstep = rs_ // exp_rsplit
                    for h in range(exp_rsplit):
                        nc.scalar.activation(
                            e3[:, :, h * rstep:(h + 1) * rstep],
                            x3[:, :, h * rstep:(h + 1) * rstep],
                            mybir.ActivationFunctionType.Exp,
                        )
                else:
                    estep = KCH * rs_ // esp
                    for d in range(esp):
                        nc.scalar.activation(
                            e[:, d * estep:(d + 1) * estep],
                            xs[:, d * estep:(d + 1) * estep],
                            mybir.ActivationFunctionType.Exp,
                        )
                u = u_pool.tile([BLK, (max(rsched) // BLK) * UST], F32, tag="u")
                if stage != "exp":
                    kch_eff = 1 if stage == "mm1" else KCH
                    for b in range(BPRS):
                        for k in range(kch_eff):
                            nc.tensor.matmul(
                                u[:, b * UST:b * UST + NA],
                                e[:, k * rs_ + b * BLK:k * rs_ + (b + 1) * BLK],
                                maug_sb[:, k * NA:(k + 1) * NA],
                                start=(k == 0),
                                stop=(k == kch_eff - 1),
                            )
                pending.append((rs_, u, r0, si))
                if len(pending) > ln_lag:
                    emit_ln(*pending.pop(0), skip_dma=compute_only)
                r0 += rs_
            for li, args in enumerate(pending):
                emit_ln(*args, skip_dma=compute_only,
                        per_block=tail_split and li == len(pending) - 1)
    try:
        nc.compile()
    finally:
        _restore_tabs()
    return nc


def make_maug2(alloW, phone_arc_labels, phoneme_arc_labels):
    """[M + (1-mrow)/P | ones] for build_graph_v2 (redistribution folded)."""
    alloW = np.asarray(alloW, dtype=np.float64).reshape(-1)
    phone = np.asarray(phone_arc_labels).astype(np.int64).reshape(-1)
    phoneme = np.asarray(phoneme_arc_labels).astype(np.int64).reshape(-1)
    M = np.zeros((C, P), dtype=np.float64)
    np.add.at(M, (phone, phoneme), np.exp(alloW))
    NA = P + 1
    maug = np.empty((C, NA), dtype=np.float64)
    maug[:, :P] = M + (1.0 - M.sum(axis=1, keepdims=True)) / P
    maug[:, P] = 1.0
    return maug.astype(np.float32).reshape(KCH, BLK, NA)


def build_empty_loop(bench_iters, n_pools=8):
    """For_i loop with a near-empty body: measures the per-iteration
    back-edge/reset overhead that every slope measurement includes."""
    from contextlib import ExitStack

    nc = bacc.Bacc("TRN2", target_bir_lowering=False, debug=False, num_devices=1)
    out_ap = nc.dram_tensor("out", [BLK, 8], F32, kind="ExternalOutput").ap()
    with tile.TileContext(nc) as tc, ExitStack() as ctx:
        pools = [ctx.enter_context(tc.tile_pool(name=f"p{i}", bufs=3))
                 for i in range(n_pools)]
        with tc.For_i(0, bench_iters, 1):
            t = pools[0].tile([BLK, 8], F32, tag="t")
            nc.vector.memset(t[:], 1.0)
            nc.sync.dma_start(out_ap[:, :], t[:])
    nc.compile()
    return nc


def build_loopcount_probe(bench_iters):
    """Tiny graph whose output literally counts loop iterations: each
    iteration DMA-accumulates a tile of ones into the output."""
    from contextlib import ExitStack

    nc = bacc.Bacc("TRN2", target_bir_lowering=False, debug=False, num_devices=1)
    out_ap = nc.dram_tensor("out", [BLK, BLK], F32, kind="ExternalOutput").ap()
    with tile.TileContext(nc) as tc, ExitStack() as ctx:
        pool = ctx.enter_context(tc.tile_pool(name="ones", bufs=1))
        ones = pool.tile([BLK, BLK], F32)
        nc.vector.memset(ones[:], 1.0)
        with tc.For_i(0, bench_iters, 1):
            nc.gpsimd.dma_start(out_ap[:, :], ones[:],
                                accum_op=mybir.AluOpType.add)
    nc.compile()
    return nc


def make_maug(alloW, phone_arc_labels, phoneme_arc_labels):
    alloW = np.asarray(alloW, dtype=np.float64).reshape(-1)
    phone = np.asarray(phone_arc_labels).astype(np.int64).reshape(-1)
    phoneme = np.asarray(phoneme_arc_labels).astype(np.int64).reshape(-1)
    M = np.zeros((C, P), dtype=np.float64)
    np.add.at(M, (phone, phoneme), np.exp(alloW))
    maug = np.empty((C, NAUG), dtype=np.float64)
    maug[:, :P] = M
    maug[:, P] = M.sum(axis=1) / P          # U[:,256] = w/P
    maug[:, P + 1] = 1.0                    # U[:,257] = s  (softmax denom)
    maug[:, P + 2] = 1.0 / P                # U[:,258] = s/P
    return maug.astype(np.float32).reshape(KCH, BLK, NAUG)


def pack_x_core(x_t, rsched, np_dtype):
    """[C, R_PER_CORE] -> [NS, BLK, KCH*rs] (each slice contiguous per
    partition: partition p holds rows {k*128+p} k-major)."""
    rs = rsched[0]
    assert all(r == rs for r in rsched)
    out = np.empty((len(rsched), BLK, KCH * rs), dtype=np_dtype)
    r0 = 0
    for si, rs_ in enumerate(rsched):
        sl = x_t[:, r0:r0 + rs_].reshape(KCH, BLK, rs_).transpose(1, 0, 2)
        out[si] = sl.reshape(BLK, KCH * rs_)
        r0 += rs_
    return out


def unpack_out_core(arr, rsched):
    """[NS, BLK, BPRS*P] device layout -> [R_PER_CORE, P] row-major."""
    rs = rsched[0]
    bprs = rs // BLK
    a = np.asarray(arr, dtype=np.float32).reshape(len(rsched), BLK, bprs, P)
    return a.transpose(0, 2, 1, 3).reshape(R_PER_CORE, P)


def make_in_maps(hs_rows_by_cols, maug, x_bf16=False):
    """Shard [ROWS, C] f32 over cores; each shard shipped transposed [C, r]."""
    xdt = ml_dtypes.bfloat16 if x_bf16 else np.float32
    return [
        {
            "x": np.ascontiguousarray(
                hs_rows_by_cols[i * R_PER_CORE:(i + 1) * R_PER_CORE].T
            ).astype(xdt),
            "maug": maug,
        }
        for i in range(N_CORES)
    ]


_NC = None


def build_best(**kw):
    fn = {"t": build_graph_t, "v2": build_graph_v2}[BEST_BUILDER]
    return fn(**BEST_CFG, **kw)


def _get_nc():
    global _NC
    if _NC is None:
        _NC = build_best()
    return _NC


def best_rsched():
    return BEST_CFG.get("rsched") or [BEST_CFG.get("rs", 512)] * (
        R_PER_CORE // BEST_CFG.get("rs", 512))


def best_x_np_dtype():
    if BEST_CFG.get("x_fp8"):
        return ml_dtypes.float8_e4m3
    if BEST_CFG.get("x_bf16", BEST_BUILDER == "v2"):
        return ml_dtypes.bfloat16
    return np.float32


def make_best_in_maps(hs_rows_by_cols, alloW, phone_arc_labels, phoneme_arc_labels):
    """Per-core input dicts for the shipped config (shard, transpose to
    [C, r], pack slices, downcast)."""
    if BEST_BUILDER == "v2":
        maug = make_maug2(alloW, phone_arc_labels, phoneme_arc_labels)
    else:
        maug = make_maug(alloW, phone_arc_labels, phoneme_arc_labels)
    xdt = best_x_np_dtype()
    in_maps = []
    for i in range(N_CORES):
        xt = np.ascontiguousarray(
            hs_rows_by_cols[i * R_PER_CORE:(i + 1) * R_PER_CORE].T)
        if BEST_CFG.get("packed_io"):
            x = pack_x_core(xt, best_rsched(), xdt)
        else:
            x = xt.astype(xdt)
        in_maps.append({"x": x, "maug": maug})
    return in_maps


def run(hs_pad, alloW, phone_arc_labels, phoneme_arc_labels, n_phonemes, trace=False):
    import time

    assert int(n_phonemes) == P
    hs = np.ascontiguousarray(np.asarray(hs_pad, dtype=np.float32)).reshape(ROWS, C)
    # data-parallel shard over rows; shards handed to the device pre-transposed
    # ([C, r] layout) so the contraction dim lands on SBUF partitions with no
    # on-chip transpose
    in_maps = make_best_in_maps(hs, alloW, phone_arc_labels, phoneme_arc_labels)
    nc = _get_nc()
    last_err = None
    for attempt in range(7):
        try:
            res = bass_utils.run_bass_kernel_spmd(
                nc, in_maps, core_ids=list(range(N_CORES)), trace=trace
            )
            break
        except Exception as e:  # transient NRT exec-unit errors recover on retry
            last_err = e
            time.sleep(min(2.0 * (attempt + 1), 10.0))
            if attempt >= 1:
                # an exec-unit-unrecoverable error wedges the PJRT client;
                # a backend reset (fresh executables) recovers where a
                # plain retry would keep failing
                try:
                    import jax
                    jax.clear_backends()
                except Exception:
                    pass
    else:
        raise last_err
    outs = []
    for i in range(N_CORES):
        o = res.results[i]["out"]
        if BEST_CFG.get("packed_io"):
            o = unpack_out_core(o, best_rsched())
        outs.append(np.asarray(o, dtype=np.float32).reshape(R_PER_CORE, P))
    return np.concatenate(outs, axis=0).reshape(B, T, P), res


def kernel(hs_pad, alloW, phone_arc_labels, phoneme_arc_labels, n_phonemes):
    out, _ = run(hs_pad, alloW, phone_arc_labels, phoneme_arc_labels, n_phonemes)
    return out


# revision 47
# speedup vs baseline: 1.1131x; 1.1131x over previous
"""AlloLayer forward on 8 TRN2 NeuronCores.

Math: reference computes
    lp   = log_softmax(hs, -1)                      # [B,T,C]
    ex   = exp(lp[..., phone_lab] + alloW)          # [B,T,A]
    sq   = scatter_add(ex, phoneme_lab)             # [B,T,P]
    red  = (sq.sum(-1) - 1) / P
    out  = log(sq - red)

The gather+exp+scatter collapses into a matmul: with
    M[c,p] = sum_{a: phone_lab[a]==c, phoneme_lab[a]==p} exp(alloW[a])
we have sq = softmax(hs) @ M.  Fold the redistribution term into the matrix,
    M'[c,p] = M[c,p] + (1 - sum_q M[c,q]) / P,      maug = [M' | ones]
so with U = exp(X) @ maug (PE, fp8/bf16 operands, f32 PSUM accumulate):
    numer = U[:,0:256]   s = U[:,256]   out = Ln(numer) - ln(s)
i.e. ONE batched ACT Ln per 512-row slice (covers numer and s of all four
128-row blocks via a 512-float-strided PSUM read), then a per-block DVE
tensor_scalar subtract that also downcasts to the bf16 output.

Data-parallel over B*T rows: 16384 rows -> 2048 rows per core, no cross-core
communication; output gathered on host.  Per-core shard is shipped fp8-e4m3,
pre-transposed AND pre-packed per 512-row slice ([128, 4*rs] with the
contraction dim on SBUF partitions, k-major per partition) so each slice is
ONE fully-contiguous input DMA and exp output feeds the matmul as lhsT with
no on-chip transpose.  Outputs are written in the device-natural packed
layout and inverse-permuted on the host.  maug is precomputed on host.

Engine budget per core (~best understood): ACT is the wall (exp 8192 +
Ln ~4112 elems/partition ~= 12us busy); PE ~7us (fp8 runs at bf16 rate
without DoubleRow), warmed up during the DMA fill by dummy matmuls so its
clock ramp (0.65/1.2/2.4 GHz p-states) completes before real work; DMA
~7.5us span (fp8 in + bf16 out = 2MB/core).  Ln work for slice i is
emitted one slice behind exp (ln_lag) so the in-order ACT stream never
stalls on the PE.
"""

import numpy as np
import ml_dtypes

import concourse.bass as bass
import concourse.tile as tile
from concourse import bacc, mybir
from concourse import bass_utils

F32 = mybir.dt.float32
BF16 = mybir.dt.bfloat16

N_CORES = 8
B, T, C, A, P = 16, 1024, 512, 4096, 256
ROWS = B * T                      # 16384
R_PER_CORE = ROWS // N_CORES      # 2048
NAUG = P + 3                      # 259: [M | (M@1)/P | ones | ones/P]
BLK = 128
KCH = C // BLK                    # 4 contraction chunks

# single source of truth for the shipped configuration (used by run() and
# by test.py's timing section).  BEST_BUILDER picks the graph builder;
# BEST_CFG are its kwargs.
# x stays bf16 (not fp8): fp8-e4m3 input quantization leaves rel_l2 at
# 3.5e-3 but pushes the max ELEMENTWISE rel err to ~5e-2, past the 2e-2
# correctness gate if it is applied per element; bf16 keeps both metrics
# comfortably inside for ~1us of extra input-DMA time.
BEST_BUILDER = "v2"
BEST_CFG = dict(x_bf16=True, packed_io=True, ln_lag=1, pe_warm=14)


def _pin_act_table(arch):
    """Make natural_log_exp_and_others the only table-set advertising Exp/Ln.

    The compiled NEFF then keeps one resident ACT table set for the whole
    kernel instead of reloading (~1.3us each) on every Exp<->Ln alternation.
    Temporarily mutates the functools-cached dict (set indices unchanged; the
    real HW set genuinely contains both functions, so execution is
    unaffected); returns a restore() closure to undo it after compile.
    """
    from concourse import hw_specs

    tabs = hw_specs.get_activation_tables(arch)
    both = "natural_log_exp_and_others"
    assert both in tabs
    af = mybir.ActivationFunctionType
    assert af.Exp in tabs[both] and af.Ln in tabs[both]
    removed = []
    for name, fns in tabs.items():
        if name != both:
            for f in (af.Exp, af.Ln):
                if f in fns:
                    fns.discard(f)
                    removed.append((fns, f))

    def restore():
        for fns, f in removed:
            fns.add(f)

    return restore


def build_graph_t(x_bufs=3, e_bufs=3, o_bufs=3, u_bufs=6, out_batch=1,
                  rs=512, in_split=2, exp_split=1, out_eng="sync",
                  in_eng="sync", out_blocks=64, exp_mode="k", maug_bf16=False,
                  ln_batch=1, x_bf16=False, out_bf16=False, rsched=None,
                  bench_iters=0, skip_mm=False, skip_dve=False,
                  dma_only=False, compute_only=False):
    """Kernel taking the per-core x shard PRE-TRANSPOSED on the host:
    x_t[C, R_PER_CORE].  No on-chip transposes: DMA loads [128c, r] tiles
    directly, exp runs on big tiles, PE does only the matmuls.

    dma_only: skip all compute; out tiles only memset and streamed out.
    compute_only: skip both in- and out-DMA (xs memset instead).
    """
    from contextlib import ExitStack, nullcontext

    nc = bacc.Bacc("TRN2", target_bir_lowering=False, debug=False, num_devices=1)
    _restore_tabs = _pin_act_table(nc.m.arch)
    x_dt = BF16 if x_bf16 else F32
    out_dt = BF16 if out_bf16 else F32
    x_ap = nc.dram_tensor("x", [C, R_PER_CORE], x_dt, kind="ExternalInput").ap()
    maug_dt = BF16 if maug_bf16 else F32
    maug_ap = nc.dram_tensor("maug", [KCH, BLK, NAUG], maug_dt, kind="ExternalInput").ap()
    out_ap = nc.dram_tensor("out", [R_PER_CORE, P], out_dt, kind="ExternalOutput").ap()

    if rsched is None:
        rsched = [rs] * (R_PER_CORE // rs)
    assert sum(rsched) == R_PER_CORE

    with tile.TileContext(nc) as tc, ExitStack() as ctx:
        const_pool = ctx.enter_context(tc.tile_pool(name="const", bufs=1))
        x_pool = ctx.enter_context(tc.tile_pool(name="xin", bufs=x_bufs))
        e_pool = ctx.enter_context(tc.tile_pool(name="e", bufs=e_bufs))
        o_pool = ctx.enter_context(tc.tile_pool(name="o", bufs=o_bufs))
        s_pool = ctx.enter_context(tc.tile_pool(name="small", bufs=4))
        v_pool = ctx.enter_context(tc.tile_pool(name="v", bufs=3))
        u_pool = ctx.enter_context(tc.tile_pool(name="u", bufs=u_bufs, space="PSUM"))

        maug_sb = const_pool.tile([BLK, KCH * NAUG], BF16)
        nc.gpsimd.dma_start(
            maug_sb[:].rearrange("p (k n) -> p k n", n=NAUG),
            maug_ap[:, :, :].rearrange("k p n -> p k n"),
        )

        loop_cm = (
            tc.For_i(0, abs(bench_iters), 1) if bench_iters else nullcontext()
        )
        ctx.enter_context(loop_cm)

        x_t3 = x_ap.rearrange("(k p) r -> k p r", p=BLK)   # [KCH, 128, R]
        kper = KCH // in_split                              # c-chunks per in-DMA
        RSMAX = max(rsched)
        r0 = 0
        for rs in rsched:
            BPRS = rs // BLK
            # x slice [128, KCH*rs]: c-chunk k occupies cols [k*rs, (k+1)*rs)
            xs = x_pool.tile([BLK, KCH * RSMAX], x_dt, tag="xs")
            if compute_only:
                nc.vector.memset(xs[:, 0:8], 0.0)
            else:
                for d in range(in_split):
                    deng = nc.sync if (in_eng == "sync" or d % 2 == 0) else nc.scalar
                    deng.dma_start(
                        xs[:, d * kper * rs:(d + 1) * kper * rs].rearrange(
                            "p (k r) -> p k r", r=rs
                        ),
                        x_t3[d * kper:(d + 1) * kper, :, r0:r0 + rs].rearrange(
                            "k p r -> p k r"
                        ),
                    )
            outs = o_pool.tile([BLK, (RSMAX // BLK) * P], out_dt, tag="outs")
            if dma_only:
                nc.vector.memset(outs[:, 0:8], 0.0)
            else:
                e = e_pool.tile([BLK, KCH * RSMAX], BF16, tag="e")
                if exp_mode == "block":
                    # one exp per row-block spanning all 4 c-chunks (strided
                    # AP): each block's matmuls wait on ONE exp, not all
                    x3 = xs[:, 0:KCH * rs].rearrange("p (k r) -> p k r", r=rs)
                    e3 = e[:, 0:KCH * rs].rearrange("p (k r) -> p k r", r=rs)
                    for b in range(rs // BLK):
                        nc.scalar.activation(
                            e3[:, :, b * BLK:(b + 1) * BLK],
                            x3[:, :, b * BLK:(b + 1) * BLK],
                            mybir.ActivationFunctionType.Exp,
                        )
                else:
                    estep = KCH * rs // exp_split
                    for s in range(exp_split):
                        nc.scalar.activation(
                            e[:, s * estep:(s + 1) * estep],
                            xs[:, s * estep:(s + 1) * estep],
                            mybir.ActivationFunctionType.Exp,
                        )
                if skip_mm:
                    nc.vector.memset(outs[:, 0:8], 0.0)
            for b in range(BPRS if not (skip_mm or dma_only) else 0):
                u = u_pool.tile([BLK, NAUG], F32, tag="u")
                for k in range(KCH):
                    nc.tensor.matmul(
                        u[:],
                        e[:, k * rs + b * BLK:k * rs + (b + 1) * BLK],
                        maug_sb[:, k * NAUG:(k + 1) * NAUG],
                        start=(k == 0),
                        stop=(k == KCH - 1),
                    )
                if skip_dve:
                    nc.scalar.activation(
                        outs[:, b * P:(b + 1) * P],
                        u[:, 0:P],
                        mybir.ActivationFunctionType.Ln,
                        bias=0.0,
                        scale=1.0,
                    )
                    continue
                inv_s = s_pool.tile([BLK, 1], F32, tag="inv")
                nc.vector.reciprocal(inv_s[:], u[:, P + 1:P + 2])
                bias_t = s_pool.tile([BLK, 1], F32, tag="bias")
                if ln_batch > 1:
                    # normalize on DVE (per-partition scalars), then one Ln
                    # per ln_batch blocks — fewer serial ACT instructions
                    if b % ln_batch == 0:
                        v = v_pool.tile([BLK, ln_batch * P], F32, tag="v")
                    # bias2 = s/P - w/P = (s - w)/P
                    nc.vector.tensor_scalar(
                        bias_t[:],
                        u[:, P + 2:P + 3],
                        u[:, P:P + 1],
                        None,
                        op0=mybir.AluOpType.subtract,
                    )
                    # V = (U + bias2) * inv_s
                    nc.vector.tensor_scalar(
                        v[:, (b % ln_batch) * P:(b % ln_batch + 1) * P],
                        u[:, 0:P],
                        bias_t[:],
                        inv_s[:],
                        op0=mybir.AluOpType.add,
                        op1=mybir.AluOpType.mult,
                    )
                    if (b + 1) % ln_batch == 0 or b == BPRS - 1:
                        g0 = (b // ln_batch) * ln_batch
                        ng = b - g0 + 1
                        nc.scalar.activation(
                            outs[:, g0 * P:(g0 + ng) * P],
                            v[:, 0:ng * P],
                            mybir.ActivationFunctionType.Ln,
                        )
                else:
                    # bias = (s/P - w/P) * (1/s) = (s - w)/(P*s)
                    nc.vector.scalar_tensor_tensor(
                        bias_t[:],
                        u[:, P + 2:P + 3],
                        u[:, P:P + 1],
                        inv_s[:],
                        op0=mybir.AluOpType.subtract,
                        op1=mybir.AluOpType.mult,
                    )
                    nc.scalar.activation(
                        outs[:, b * P:(b + 1) * P],
                        u[:, 0:P],
                        mybir.ActivationFunctionType.Ln,
                        bias=bias_t[:],
                        scale=inv_s[:],
                    )
            if not compute_only:
                for b0 in range(0, BPRS, out_blocks):
                    nb = min(out_blocks, BPRS - b0)
                    getattr(nc, out_eng).dma_start(
                        out_ap[r0 + b0 * BLK:r0 + (b0 + nb) * BLK, :].rearrange(
                            "(b p) c -> p b c", p=BLK
                        ),
                        outs[:, b0 * P:(b0 + nb) * P].rearrange(
                            "p (b c) -> p b c", c=P
                        ),
                    )
            r0 += rs
    try:
        nc.compile()
    finally:
        _restore_tabs()
    return nc


def build_graph_v2(rsched=None, rs=512, e_bufs=3, o_bufs=3, lnu_bufs=3,
                   out_blocks=0, out_eng="sync", in_split=1, x_bf16=True,
                   out_bf16=True, x_fp8=False, ln_lag=1, pe_warm=0,
                   packed_io=False, fill_split=1, sub_eng="vector",
                   exp_rsplit=1, tail_split=False, bench_iters=0,
                   dma_only=False, compute_only=False, stage="all"):
    """Restructured kernel:

    - maug folds the redistribution term: M'[c,p] = M[c,p] + (1-mrow[c])/P,
      plus a ones column -> U = e @ [M' | 1] gives numer and s directly and
      out = Ln(numer) - ln(s); the Ln needs no per-block scale/bias, so it
      batches across a whole slice.
    - per-slice PSUM tile [128, BPRS*512] f32 (one bank per 128-row block,
      [numer | s] in cols [b*512, b*512+257)); ONE strided ACT Ln per slice
      covers numer and s of every block.
    - ln(s) subtract + bf16 downcast on DVE, per block.
    - all in-DMAs emitted first (no head-of-line blocking on SP), one DMA
      per slice; outputs bf16.
    - ln_lag: slices of Ln/subtract/out work are emitted that many slices
      behind the exp/matmul front, so the in-order ACT stream never stalls
      waiting on a cold PE.
    """
    from contextlib import ExitStack, nullcontext

    nc = bacc.Bacc("TRN2", target_bir_lowering=False, debug=False, num_devices=1)
    _restore_tabs = _pin_act_table(nc.m.arch)
    x_dt = mybir.dt.float8e4 if x_fp8 else (BF16 if x_bf16 else F32)
    out_dt = BF16 if out_bf16 else F32
    NA = P + 1                 # 257: [M' | ones]
    UST = 512                  # psum floats per block (bank-aligned)
    if rsched is None:
        rsched = [rs] * (R_PER_CORE // rs)
    assert sum(rsched) == R_PER_CORE
    if packed_io:
        # host pre-permutes x so each slice is one fully contiguous
        # [128, KCH*rs] block (per-partition DMA runs of KCH*rs elements
        # instead of rs), and the output is written in the device-natural
        # [slice][p, blk*c] order (host inverse-permutes after gather);
        # uniform slices only
        assert all(r == rsched[0] for r in rsched)
        x_ap = nc.dram_tensor("x", [len(rsched), BLK, KCH * rsched[0]], x_dt,
                              kind="ExternalInput").ap()
        out_ap = nc.dram_tensor("out", [len(rsched), BLK, (rsched[0] // BLK) * P],
                                out_dt, kind="ExternalOutput").ap()
    else:
        x_ap = nc.dram_tensor("x", [C, R_PER_CORE], x_dt, kind="ExternalInput").ap()
        out_ap = nc.dram_tensor("out", [R_PER_CORE, P], out_dt,
                                kind="ExternalOutput").ap()
    maug_ap = nc.dram_tensor("maug", [KCH, BLK, NA], F32, kind="ExternalInput").ap()
    NS = len(rsched)
    # live slice-level PSUM tiles (ring slots sized by the largest slice)
    # must fit in the 8 PSUM banks
    u_bufs = max(2, ln_lag + 1)
    assert u_bufs * max(rsched) <= 1024

    with tile.TileContext(nc) as tc, ExitStack() as ctx:
        const_pool = ctx.enter_context(tc.tile_pool(name="const", bufs=1))
        x_pool = ctx.enter_context(tc.tile_pool(name="xin", bufs=NS))
        e_pool = ctx.enter_context(tc.tile_pool(name="e", bufs=e_bufs))
        lnu_pool = ctx.enter_context(tc.tile_pool(name="lnu", bufs=lnu_bufs))
        o_pool = ctx.enter_context(tc.tile_pool(name="o", bufs=o_bufs))
        s_pool = ctx.enter_context(tc.tile_pool(name="small", bufs=4))
        u_pool = ctx.enter_context(tc.tile_pool(name="u", bufs=u_bufs, space="PSUM"))

        maug_sb = const_pool.tile([BLK, KCH * NA], BF16)
        nc.gpsimd.dma_start(
            maug_sb[:].rearrange("p (k n) -> p k n", n=NA),
            maug_ap[:, :, :].rearrange("k p n -> p k n"),
        )
        zwarm = None
        if pe_warm:
            zwarm = const_pool.tile([BLK, 2 * BLK], BF16)
            nc.vector.memset(zwarm[:], 0.0)

        loop_cm = (
            tc.For_i(0, abs(bench_iters), 1) if bench_iters else nullcontext()
        )
        ctx.enter_context(loop_cm)

        if pe_warm and not dma_only:
            # dummy matmuls keep the PE p-state ramp going during the DMA
            # fill window so the first real matmuls run at full clock
            # (borrows a u-pool slot; freed before slice 1 needs it)
            wu = u_pool.tile([BLK, (max(rsched) // BLK) * UST], F32, tag="u")
            for i in range(pe_warm):
                nc.tensor.matmul(
                    wu[:, 0:2 * BLK], zwarm[:, 0:BLK], zwarm[:, 0:2 * BLK],
                    start=True, stop=True,
                )

        x_t3 = None if packed_io else x_ap.rearrange("(k p) r -> k p r", p=BLK)
        RSMAX = max(rsched)
        kper = KCH // in_split

        # phase A: all input DMAs, one (or in_split) per slice, on SP HWDGE
        xs_tiles = []
        r0 = 0
        for si, rs_ in enumerate(rsched):
            xs = x_pool.tile([BLK, KCH * RSMAX], x_dt, tag="xs")
            if compute_only:
                nc.vector.memset(xs[:, 0:8], 0.0)
            elif packed_io:
                isp = fill_split if (si == 0 and fill_split > 1) else in_split
                step = KCH * rs_ // isp
                for d in range(isp):
                    nc.sync.dma_start(
                        xs[:, d * step:(d + 1) * step],
                        x_ap[si, :, d * step:(d + 1) * step],
                    )
            else:
                for d in range(in_split):
                    nc.sync.dma_start(
                        xs[:, d * kper * rs_:(d + 1) * kper * rs_].rearrange(
                            "p (k r) -> p k r", r=rs_
                        ),
                        x_t3[d * kper:(d + 1) * kper, :, r0:r0 + rs_].rearrange(
                            "k p r -> p k r"
                        ),
                    )
            xs_tiles.append(xs)
            r0 += rs_

        # phase B: per slice: exp -> matmuls; Ln/subtract/out trail ln_lag
        # slices behind so the in-order ACT stream never waits on the PE.
        NAW = NA  # columns per block read by the batched Ln ([numer | s])

        def emit_out_dma(BPRS, si_, r0_, outs):
            ob = out_blocks if out_blocks else BPRS
            for b0 in range(0, BPRS, ob):
                nb = min(ob, BPRS - b0)
                if packed_io:
                    getattr(nc, out_eng).dma_start(
                        out_ap[si_, :, b0 * P:(b0 + nb) * P],
                        outs[:, b0 * P:(b0 + nb) * P],
                    )
                else:
                    getattr(nc, out_eng).dma_start(
                        out_ap[r0_ + b0 * BLK:r0_ + (b0 + nb) * BLK, :].rearrange(
                            "(b p) c -> p b c", p=BLK
                        ),
                        outs[:, b0 * P:(b0 + nb) * P].rearrange(
                            "p (b c) -> p b c", c=P
                        ),
                    )

        def emit_ln(rs_, u, r0_, si_, skip_dma=False, per_block=False):
            if stage in ("exp", "mm", "mm1"):
                return
            BPRS = rs_ // BLK
            outs = o_pool.tile([BLK, (RSMAX // BLK) * P], out_dt, tag="outs")
            u3 = u[:, 0:BPRS * UST].rearrange("p (b x) -> p b x", x=UST)
            lnu = lnu_pool.tile([BLK, (RSMAX // BLK) * NAW], F32, tag="lnu")
            lnu3 = lnu[:, 0:BPRS * NAW].rearrange("p (b x) -> p b x", x=NAW)
            if not per_block:
                # one ACT op: ln of numer AND s for every block of the slice
                nc.scalar.activation(
                    lnu3[:, :, :], u3[:, :, 0:NAW],
                    mybir.ActivationFunctionType.Ln,
                )
            if stage == "ln" and not per_block:
                return
            for b in range(BPRS):
                if per_block:
                    # drain mode: Ln -> subtract -> out-DMA pipelined per
                    # block so the kernel tail is short
                    nc.scalar.activation(
                        lnu3[:, b:b + 1, :], u3[:, b:b + 1, 0:NAW],
                        mybir.ActivationFunctionType.Ln,
                    )
                seng = nc.vector if (sub_eng == "vector" or b % 2 == 0) else nc.gpsimd
                seng.tensor_scalar(
                    outs[:, b * P:(b + 1) * P],
                    lnu[:, b * NAW:b * NAW + P],
                    lnu[:, b * NAW + P:b * NAW + P + 1],
                    None,
                    op0=mybir.AluOpType.subtract,
                )
                if per_block and not skip_dma:
                    if packed_io:
                        getattr(nc, out_eng).dma_start(
                            out_ap[si_, :, b * P:(b + 1) * P],
                            outs[:, b * P:(b + 1) * P],
                        )
                    else:
                        getattr(nc, out_eng).dma_start(
                            out_ap[r0_ + b * BLK:r0_ + (b + 1) * BLK, :].rearrange(
                                "(o p) c -> p o c", p=BLK
                            ),
                            outs[:, b * P:(b + 1) * P].rearrange(
                                "p (o c) -> p o c", c=P
                            ),
                        )
            if skip_dma or per_block:
                return
            emit_out_dma(BPRS, si_, r0_, outs)

        if dma_only:
            r0 = 0
            for si, rs_ in enumerate(rsched):
                BPRS = rs_ // BLK
                outs = o_pool.tile([BLK, (RSMAX // BLK) * P], out_dt, tag="outs")
                nc.vector.memset(outs[:, 0:8], 0.0)
                emit_out_dma(BPRS, si, r0, outs)
                r0 += rs_
        else:
            pending = []
            r0 = 0
            for si, rs_ in enumerate(rsched):
                BPRS = rs_ // BLK
                xs = xs_tiles[si]
                e = e_pool.tile([BLK, KCH * RSMAX], BF16, tag="e")
                esp = fill_split if (si == 0 and fill_split > 1 and packed_io
                                     and not compute_only) else 1
                if exp_rsplit > 1 and esp == 1:
                    # split the slice's exp by ROW halves (strided 3D APs):
                    # the first blocks' matmuls start after half the exp
                    x3 = xs[:, 0:KCH * rs_].rearrange("p (k r) -> p k r", r=rs_)
                    e3 = e[:, 0:KCH * rs_].rearrange("p (k r) -> p k r", r=rs_)
                    rstep = rs_ // exp_rsplit
                    for h in range(exp_rsplit):
                        nc.scalar.activation(
                            e3[:, :, h * rstep:(h + 1) * rstep],
                            x3[:, :, h * rstep:(h + 1) * rstep],
                            mybir.ActivationFunctionType.Exp,
                        )
                else:
                    estep = KCH * rs_ // esp
                    for d in range(esp):
                        nc.scalar.activation(
                            e[:, d * estep:(d + 1) * estep],
                            xs[:, d * estep:(d + 1) * estep],
                            mybir.ActivationFunctionType.Exp,
                        )
                u = u_pool.tile([BLK, (max(rsched) // BLK) * UST], F32, tag="u")
                if stage != "exp":
                    kch_eff = 1 if stage == "mm1" else KCH
                    for b in range(BPRS):
                        for k in range(kch_eff):
                            nc.tensor.matmul(
                                u[:, b * UST:b * UST + NA],
                                e[:, k * rs_ + b * BLK:k * rs_ + (b + 1) * BLK],
                                maug_sb[:, k * NA:(k + 1) * NA],
                                start=(k == 0),
                                stop=(k == kch_eff - 1),
                            )
                pending.append((rs_, u, r0, si))
                if len(pending) > ln_lag:
                    emit_ln(*pending.pop(0), skip_dma=compute_only)
                r0 += rs_
            for li, args in enumerate(pending):
                emit_ln(*args, skip_dma=compute_only,
                        per_block=tail_split and li == len(pending) - 1)
    try:
        nc.compile()
    finally:
        _restore_tabs()
    return nc


def make_maug2(alloW, phone_arc_labels, phoneme_arc_labels):
    """[M + (1-mrow)/P | ones] for build_graph_v2 (redistribution folded)."""
    alloW = np.asarray(alloW, dtype=np.float64).reshape(-1)
    phone = np.asarray(phone_arc_labels).astype(np.int64).reshape(-1)
    phoneme = np.asarray(phoneme_arc_labels).astype(np.int64).reshape(-1)
    M = np.zeros((C, P), dtype=np.float64)
    np.add.at(M, (phone, phoneme), np.exp(alloW))
    NA = P + 1
    maug = np.empty((C, NA), dtype=np.float64)
    maug[:, :P] = M + (1.0 - M.sum(axis=1, keepdims=True)) / P
    maug[:, P] = 1.0
    return maug.astype(np.float32).reshape(KCH, BLK, NA)


def build_empty_loop(bench_iters, n_pools=8):
    """For_i loop with a near-empty body: measures the per-iteration
    back-edge/reset overhead that every slope measurement includes."""
    from contextlib import ExitStack

    nc = bacc.Bacc("TRN2", target_bir_lowering=False, debug=False, num_devices=1)
    out_ap = nc.dram_tensor("out", [BLK, 8], F32, kind="ExternalOutput").ap()
    with tile.TileContext(nc) as tc, ExitStack() as ctx:
        pools = [ctx.enter_context(tc.tile_pool(name=f"p{i}", bufs=3))
                 for i in range(n_pools)]
        with tc.For_i(0, bench_iters, 1):
            t = pools[0].tile([BLK, 8], F32, tag="t")
            nc.vector.memset(t[:], 1.0)
            nc.sync.dma_start(out_ap[:, :], t[:])
    nc.compile()
    return nc


def build_loopcount_probe(bench_iters):
    """Tiny graph whose output literally counts loop iterations: each
    iteration DMA-accumulates a tile of ones into the output."""
    from contextlib import ExitStack

    nc = bacc.Bacc("TRN2", target_bir_lowering=False, debug=False, num_devices=1)
    out_ap = nc.dram_tensor("out", [BLK, BLK], F32, kind="ExternalOutput").ap()
    with tile.TileContext(nc) as tc, ExitStack() as ctx:
        pool = ctx.enter_context(tc.tile_pool(name="ones", bufs=1))
        ones = pool.tile([BLK, BLK], F32)
        nc.vector.memset(ones[:], 1.0)
        with tc.For_i(0, bench_iters, 1):
            nc.gpsimd.dma_start(out_ap[:, :], ones[:],
                                accum_op=mybir.AluOpType.add)
    nc.compile()
    return nc


def make_maug(alloW, phone_arc_labels, phoneme_arc_labels):
    alloW = np.asarray(alloW, dtype=np.float64).reshape(-1)
    phone = np.asarray(phone_arc_labels).astype(np.int64).reshape(-1)
    phoneme = np.asarray(phoneme_arc_labels).astype(np.int64).reshape(-1)
    M = np.zeros((C, P), dtype=np.float64)
    np.add.at(M, (phone, phoneme), np.exp(alloW))
    maug = np.empty((C, NAUG), dtype=np.float64)
    maug[:, :P] = M
    maug[:, P] = M.sum(axis=1) / P          # U[:,256] = w/P
    maug[:, P + 1] = 1.0                    # U[:,257] = s  (softmax denom)
    maug[:, P + 2] = 1.0 / P                # U[:,258] = s/P
    return maug.astype(np.float32).reshape(KCH, BLK, NAUG)


def pack_x_core(x_t, rsched, np_dtype):
    """[C, R_PER_CORE] -> [NS, BLK, KCH*rs] (each slice contiguous per
    partition: partition p holds rows {k*128+p} k-major)."""
    rs = rsched[0]
    assert all(r == rs for r in rsched)
    out = np.empty((len(rsched), BLK, KCH * rs), dtype=np_dtype)
    r0 = 0
    for si, rs_ in enumerate(rsched):
        sl = x_t[:, r0:r0 + rs_].reshape(KCH, BLK, rs_).transpose(1, 0, 2)
        out[si] = sl.reshape(BLK, KCH * rs_)
        r0 += rs_
    return out


def unpack_out_core(arr, rsched):
    """[NS, BLK, BPRS*P] device layout -> [R_PER_CORE, P] row-major."""
    rs = rsched[0]
    bprs = rs // BLK
    a = np.asarray(arr, dtype=np.float32).reshape(len(rsched), BLK, bprs, P)
    return a.transpose(0, 2, 1, 3).reshape(R_PER_CORE, P)


def make_in_maps(hs_rows_by_cols, maug, x_bf16=False):
    """Shard [ROWS, C] f32 over cores; each shard shipped transposed [C, r]."""
    xdt = ml_dtypes.bfloat16 if x_bf16 else np.float32
    return [
        {
            "x": np.ascontiguousarray(
                hs_rows_by_cols[i * R_PER_CORE:(i + 1) * R_PER_CORE].T
            ).astype(xdt),
            "maug": maug,
        }
        for i in range(N_CORES)
    ]


_NC = None


def build_best(**kw):
    fn = {"t": build_graph_t, "v2": build_graph_v2}[BEST_BUILDER]
    return fn(**BEST_CFG, **kw)


def _get_nc():
    global _NC
    if _NC is None:
        _NC = build_best()
    return _NC


def best_rsched():
    return BEST_CFG.get("rsched") or [BEST_CFG.get("rs", 512)] * (
        R_PER_CORE // BEST_CFG.get("rs", 512))


def best_x_np_dtype():
    if BEST_CFG.get("x_fp8"):
        return ml_dtypes.float8_e4m3
    if BEST_CFG.get("x_bf16", BEST_BUILDER == "v2"):
        return ml_dtypes.bfloat16
    return np.float32


def make_best_in_maps(hs_rows_by_cols, alloW, phone_arc_labels, phoneme_arc_labels):
    """Per-core input dicts for the shipped config (shard, transpose to
    [C, r], pack slices, downcast)."""
    if BEST_BUILDER == "v2":
        maug = make_maug2(alloW, phone_arc_labels, phoneme_arc_labels)
    else:
        maug = make_maug(alloW, phone_arc_labels, phoneme_arc_labels)
    xdt = best_x_np_dtype()
    in_maps = []
    for i in range(N_CORES):
        xt = np.ascontiguousarray(
            hs_rows_by_cols[i * R_PER_CORE:(i + 1) * R_PER_CORE].T)
        if BEST_CFG.get("packed_io"):
            x = pack_x_core(xt, best_rsched(), xdt)
        else:
            x = xt.astype(xdt)
        in_maps.append({"x": x, "maug": maug})
    return in_maps


def run(hs_pad, alloW, phone_arc_labels, phoneme_arc_labels, n_phonemes, trace=False):
    import time

    assert int(n_phonemes) == P
    hs = np.ascontiguousarray(np.asarray(hs_pad, dtype=np.float32)).reshape(ROWS, C)
    # data-parallel shard over rows; shards handed to the device pre-transposed
    # ([C, r] layout) so the contraction dim lands on SBUF partitions with no
    # on-chip transpose
    in_maps = make_best_in_maps(hs, alloW, phone_arc_labels, phoneme_arc_labels)
    nc = _get_nc()
    last_err = None
    for attempt in range(7):
        try:
            res = bass_utils.run_bass_kernel_spmd(
                nc, in_maps, core_ids=list(range(N_CORES)), trace=trace
            )
            break
        except Exception as e:  # transient NRT exec-unit errors recover on retry
            last_err = e
            time.sleep(min(2.0 * (attempt + 1), 10.0))
            if attempt >= 1:
                # an exec-unit-unrecoverable error wedges the PJRT client;
                # a backend reset (fresh executables) recovers where a
                # plain retry would keep failing
                try:
                    import jax
                    jax.clear_backends()
                except Exception:
                    pass
    else:
        raise last_err
    outs = []
    for i in range(N_CORES):
        o = res.results[i]["out"]
        if BEST_CFG.get("packed_io"):
            o = unpack_out_core(o, best_rsched())
        outs.append(np.asarray(o, dtype=np.float32).reshape(R_PER_CORE, P))
    return np.concatenate(outs, axis=0).reshape(B, T, P), res


def kernel(hs_pad, alloW, phone_arc_labels, phoneme_arc_labels, n_phonemes):
    out, _ = run(hs_pad, alloW, phone_arc_labels, phoneme_arc_labels, n_phonemes)
    return out


# revision 50
# speedup vs baseline: 1.1698x; 1.0509x over previous
"""AlloLayer forward on 8 TRN2 NeuronCores.

Math: reference computes
    lp   = log_softmax(hs, -1)                      # [B,T,C]
    ex   = exp(lp[..., phone_lab] + alloW)          # [B,T,A]
    sq   = scatter_add(ex, phoneme_lab)             # [B,T,P]
    red  = (sq.sum(-1) - 1) / P
    out  = log(sq - red)

The gather+exp+scatter collapses into a matmul: with
    M[c,p] = sum_{a: phone_lab[a]==c, phoneme_lab[a]==p} exp(alloW[a])
we have sq = softmax(hs) @ M.  Fold the redistribution term into the matrix,
    M'[c,p] = M[c,p] + (1 - sum_q M[c,q]) / P,      maug = [M' | ones]
so with U = exp(X) @ maug (PE, fp8/bf16 operands, f32 PSUM accumulate):
    numer = U[:,0:256]   s = U[:,256]   out = Ln(numer) - ln(s)
i.e. ONE batched ACT Ln per 512-row slice (covers numer and s of all four
128-row blocks via a 512-float-strided PSUM read), then a per-block DVE
tensor_scalar subtract that also downcasts to the bf16 output.

Data-parallel over B*T rows: 16384 rows -> 2048 rows per core, no cross-core
communication; output gathered on host.  Per-core shard is shipped fp8-e4m3,
pre-transposed AND pre-packed per 512-row slice ([128, 4*rs] with the
contraction dim on SBUF partitions, k-major per partition) so each slice is
ONE fully-contiguous input DMA and exp output feeds the matmul as lhsT with
no on-chip transpose.  Outputs are written in the device-natural packed
layout and inverse-permuted on the host.  maug is precomputed on host.

Engine budget per core (~best understood): ACT is the wall (exp 8192 +
Ln ~4112 elems/partition ~= 12us busy); PE ~7us (fp8 runs at bf16 rate
without DoubleRow), warmed up during the DMA fill by dummy matmuls so its
clock ramp (0.65/1.2/2.4 GHz p-states) completes before real work; DMA
~7.5us span (fp8 in + bf16 out = 2MB/core).  Ln work for slice i is
emitted one slice behind exp (ln_lag) so the in-order ACT stream never
stalls on the PE.
"""

import numpy as np
import ml_dtypes

import concourse.bass as bass
import concourse.tile as tile
from concourse import bacc, mybir
from concourse import bass_utils

F32 = mybir.dt.float32
BF16 = mybir.dt.bfloat16

N_CORES = 8
B, T, C, A, P = 16, 1024, 512, 4096, 256
ROWS = B * T                      # 16384
R_PER_CORE = ROWS // N_CORES      # 2048
NAUG = P + 3                      # 259: [M | (M@1)/P | ones | ones/P]
BLK = 128
KCH = C // BLK                    # 4 contraction chunks

# single source of truth for the shipped configuration (used by run() and
# by test.py's timing section).  BEST_BUILDER picks the graph builder;
# BEST_CFG are its kwargs.
# x stays bf16 (not fp8): fp8-e4m3 input quantization leaves rel_l2 at
# 3.5e-3 but pushes the max ELEMENTWISE rel err to ~5e-2, past the 2e-2
# correctness gate if it is applied per element; bf16 keeps both metrics
# comfortably inside for ~1us of extra input-DMA time.
# rsched ramps DOWN at the end: the final slice's serial drain chain
# (PE -> Ln -> subtract -> out-DMA) scales with the last slice size, so
# small last slices shorten the kernel tail (~0.8us); a small FIRST slice
# measured worse (exp outruns the input-DMA stream early).
BEST_BUILDER = "v2"
BEST_CFG = dict(x_bf16=True, packed_io=True, ln_lag=1, pe_warm=14,
                rsched=[512, 512, 512, 384, 128])


def _pin_act_table(arch):
    """Make natural_log_exp_and_others the only table-set advertising Exp/Ln.

    The compiled NEFF then keeps one resident ACT table set for the whole
    kernel instead of reloading (~1.3us each) on every Exp<->Ln alternation.
    Temporarily mutates the functools-cached dict (set indices unchanged; the
    real HW set genuinely contains both functions, so execution is
    unaffected); returns a restore() closure to undo it after compile.
    """
    from concourse import hw_specs

    tabs = hw_specs.get_activation_tables(arch)
    both = "natural_log_exp_and_others"
    assert both in tabs
    af = mybir.ActivationFunctionType
    assert af.Exp in tabs[both] and af.Ln in tabs[both]
    removed = []
    for name, fns in tabs.items():
        if name != both:
            for f in (af.Exp, af.Ln):
                if f in fns:
                    fns.discard(f)
                    removed.append((fns, f))

    def restore():
        for fns, f in removed:
            fns.add(f)

    return restore


def build_graph_t(x_bufs=3, e_bufs=3, o_bufs=3, u_bufs=6, out_batch=1,
                  rs=512, in_split=2, exp_split=1, out_eng="sync",
                  in_eng="sync", out_blocks=64, exp_mode="k", maug_bf16=False,
                  ln_batch=1, x_bf16=False, out_bf16=False, rsched=None,
                  bench_iters=0, skip_mm=False, skip_dve=False,
                  dma_only=False, compute_only=False):
    """Kernel taking the per-core x shard PRE-TRANSPOSED on the host:
    x_t[C, R_PER_CORE].  No on-chip transposes: DMA loads [128c, r] tiles
    directly, exp runs on big tiles, PE does only the matmuls.

    dma_only: skip all compute; out tiles only memset and streamed out.
    compute_only: skip both in- and out-DMA (xs memset instead).
    """
    from contextlib import ExitStack, nullcontext

    nc = bacc.Bacc("TRN2", target_bir_lowering=False, debug=False, num_devices=1)
    _restore_tabs = _pin_act_table(nc.m.arch)
    x_dt = BF16 if x_bf16 else F32
    out_dt = BF16 if out_bf16 else F32
    x_ap = nc.dram_tensor("x", [C, R_PER_CORE], x_dt, kind="ExternalInput").ap()
    maug_dt = BF16 if maug_bf16 else F32
    maug_ap = nc.dram_tensor("maug", [KCH, BLK, NAUG], maug_dt, kind="ExternalInput").ap()
    out_ap = nc.dram_tensor("out", [R_PER_CORE, P], out_dt, kind="ExternalOutput").ap()

    if rsched is None:
        rsched = [rs] * (R_PER_CORE // rs)
    assert sum(rsched) == R_PER_CORE

    with tile.TileContext(nc) as tc, ExitStack() as ctx:
        const_pool = ctx.enter_context(tc.tile_pool(name="const", bufs=1))
        x_pool = ctx.enter_context(tc.tile_pool(name="xin", bufs=x_bufs))
        e_pool = ctx.enter_context(tc.tile_pool(name="e", bufs=e_bufs))
        o_pool = ctx.enter_context(tc.tile_pool(name="o", bufs=o_bufs))
        s_pool = ctx.enter_context(tc.tile_pool(name="small", bufs=4))
        v_pool = ctx.enter_context(tc.tile_pool(name="v", bufs=3))
        u_pool = ctx.enter_context(tc.tile_pool(name="u", bufs=u_bufs, space="PSUM"))

        maug_sb = const_pool.tile([BLK, KCH * NAUG], BF16)
        nc.gpsimd.dma_start(
            maug_sb[:].rearrange("p (k n) -> p k n", n=NAUG),
            maug_ap[:, :, :].rearrange("k p n -> p k n"),
        )

        loop_cm = (
            tc.For_i(0, abs(bench_iters), 1) if bench_iters else nullcontext()
        )
        ctx.enter_context(loop_cm)

        x_t3 = x_ap.rearrange("(k p) r -> k p r", p=BLK)   # [KCH, 128, R]
        kper = KCH // in_split                              # c-chunks per in-DMA
        RSMAX = max(rsched)
        r0 = 0
        for rs in rsched:
            BPRS = rs // BLK
            # x slice [128, KCH*rs]: c-chunk k occupies cols [k*rs, (k+1)*rs)
            xs = x_pool.tile([BLK, KCH * RSMAX], x_dt, tag="xs")
            if compute_only:
                nc.vector.memset(xs[:, 0:8], 0.0)
            else:
                for d in range(in_split):
                    deng = nc.sync if (in_eng == "sync" or d % 2 == 0) else nc.scalar
                    deng.dma_start(
                        xs[:, d * kper * rs:(d + 1) * kper * rs].rearrange(
                            "p (k r) -> p k r", r=rs
                        ),
                        x_t3[d * kper:(d + 1) * kper, :, r0:r0 + rs].rearrange(
                            "k p r -> p k r"
                        ),
                    )
            outs = o_pool.tile([BLK, (RSMAX // BLK) * P], out_dt, tag="outs")
            if dma_only:
                nc.vector.memset(outs[:, 0:8], 0.0)
            else:
                e = e_pool.tile([BLK, KCH * RSMAX], BF16, tag="e")
                if exp_mode == "block":
                    # one exp per row-block spanning all 4 c-chunks (strided
                    # AP): each block's matmuls wait on ONE exp, not all
                    x3 = xs[:, 0:KCH * rs].rearrange("p (k r) -> p k r", r=rs)
                    e3 = e[:, 0:KCH * rs].rearrange("p (k r) -> p k r", r=rs)
                    for b in range(rs // BLK):
                        nc.scalar.activation(
                            e3[:, :, b * BLK:(b + 1) * BLK],
                            x3[:, :, b * BLK:(b + 1) * BLK],
                            mybir.ActivationFunctionType.Exp,
                        )
                else:
                    estep = KCH * rs // exp_split
                    for s in range(exp_split):
                        nc.scalar.activation(
                            e[:, s * estep:(s + 1) * estep],
                            xs[:, s * estep:(s + 1) * estep],
                            mybir.ActivationFunctionType.Exp,
                        )
                if skip_mm:
                    nc.vector.memset(outs[:, 0:8], 0.0)
            for b in range(BPRS if not (skip_mm or dma_only) else 0):
                u = u_pool.tile([BLK, NAUG], F32, tag="u")
                for k in range(KCH):
                    nc.tensor.matmul(
                        u[:],
                        e[:, k * rs + b * BLK:k * rs + (b + 1) * BLK],
                        maug_sb[:, k * NAUG:(k + 1) * NAUG],
                        start=(k == 0),
                        stop=(k == KCH - 1),
                    )
                if skip_dve:
                    nc.scalar.activation(
                        outs[:, b * P:(b + 1) * P],
                        u[:, 0:P],
                        mybir.ActivationFunctionType.Ln,
                        bias=0.0,
                        scale=1.0,
                    )
                    continue
                inv_s = s_pool.tile([BLK, 1], F32, tag="inv")
                nc.vector.reciprocal(inv_s[:], u[:, P + 1:P + 2])
                bias_t = s_pool.tile([BLK, 1], F32, tag="bias")
                if ln_batch > 1:
                    # normalize on DVE (per-partition scalars), then one Ln
                    # per ln_batch blocks — fewer serial ACT instructions
                    if b % ln_batch == 0:
                        v = v_pool.tile([BLK, ln_batch * P], F32, tag="v")
                    # bias2 = s/P - w/P = (s - w)/P
                    nc.vector.tensor_scalar(
                        bias_t[:],
                        u[:, P + 2:P + 3],
                        u[:, P:P + 1],
                        None,
                        op0=mybir.AluOpType.subtract,
                    )
                    # V = (U + bias2) * inv_s
                    nc.vector.tensor_scalar(
                        v[:, (b % ln_batch) * P:(b % ln_batch + 1) * P],
                        u[:, 0:P],
                        bias_t[:],
                        inv_s[:],
                        op0=mybir.AluOpType.add,
                        op1=mybir.AluOpType.mult,
                    )
                    if (b + 1) % ln_batch == 0 or b == BPRS - 1:
                        g0 = (b // ln_batch) * ln_batch
                        ng = b - g0 + 1
                        nc.scalar.activation(
                            outs[:, g0 * P:(g0 + ng) * P],
                            v[:, 0:ng * P],
                            mybir.ActivationFunctionType.Ln,
                        )
                else:
                    # bias = (s/P - w/P) * (1/s) = (s - w)/(P*s)
                    nc.vector.scalar_tensor_tensor(
                        bias_t[:],
                        u[:, P + 2:P + 3],
                        u[:, P:P + 1],
                        inv_s[:],
                        op0=mybir.AluOpType.subtract,
                        op1=mybir.AluOpType.mult,
                    )
                    nc.scalar.activation(
                        outs[:, b * P:(b + 1) * P],
                        u[:, 0:P],
                        mybir.ActivationFunctionType.Ln,
                        bias=bias_t[:],
                        scale=inv_s[:],
                    )
            if not compute_only:
                for b0 in range(0, BPRS, out_blocks):
                    nb = min(out_blocks, BPRS - b0)
                    getattr(nc, out_eng).dma_start(
                        out_ap[r0 + b0 * BLK:r0 + (b0 + nb) * BLK, :].rearrange(
                            "(b p) c -> p b c", p=BLK
                        ),
                        outs[:, b0 * P:(b0 + nb) * P].rearrange(
                            "p (b c) -> p b c", c=P
                        ),
                    )
            r0 += rs
    try:
        nc.compile()
    finally:
        _restore_tabs()
    return nc


def build_graph_v2(rsched=None, rs=512, e_bufs=3, o_bufs=3, lnu_bufs=3,
                   out_blocks=0, out_eng="sync", in_split=1, x_bf16=True,
                   out_bf16=True, x_fp8=False, ln_lag=1, pe_warm=0,
                   packed_io=False, fill_split=1, sub_eng="vector",
                   exp_rsplit=1, tail_split=False, bench_iters=0,
                   dma_only=False, compute_only=False, stage="all"):
    """Restructured kernel:

    - maug folds the redistribution term: M'[c,p] = M[c,p] + (1-mrow[c])/P,
      plus a ones column -> U = e @ [M' | 1] gives numer and s directly and
      out = Ln(numer) - ln(s); the Ln needs no per-block scale/bias, so it
      batches across a whole slice.
    - per-slice PSUM tile [128, BPRS*512] f32 (one bank per 128-row block,
      [numer | s] in cols [b*512, b*512+257)); ONE strided ACT Ln per slice
      covers numer and s of every block.
    - ln(s) subtract + bf16 downcast on DVE, per block.
    - all in-DMAs emitted first (no head-of-line blocking on SP), one DMA
      per slice; outputs bf16.
    - ln_lag: slices of Ln/subtract/out work are emitted that many slices
      behind the exp/matmul front, so the in-order ACT stream never stalls
      waiting on a cold PE.
    """
    from contextlib import ExitStack, nullcontext

    nc = bacc.Bacc("TRN2", target_bir_lowering=False, debug=False, num_devices=1)
    _restore_tabs = _pin_act_table(nc.m.arch)
    x_dt = mybir.dt.float8e4 if x_fp8 else (BF16 if x_bf16 else F32)
    out_dt = BF16 if out_bf16 else F32
    NA = P + 1                 # 257: [M' | ones]
    UST = 512                  # psum floats per block (bank-aligned)
    if rsched is None:
        rsched = [rs] * (R_PER_CORE // rs)
    assert sum(rsched) == R_PER_CORE
    if packed_io:
        # host pre-permutes x so each slice is one fully contiguous
        # [128, KCH*rs] block (per-partition DMA runs of KCH*rs elements
        # instead of rs), and the output is written in the device-natural
        # [slice][p, blk*c] order (host inverse-permutes after gather);
        # non-uniform slices are padded to the largest slice
        x_ap = nc.dram_tensor("x", [len(rsched), BLK, KCH * max(rsched)], x_dt,
                              kind="ExternalInput").ap()
        out_ap = nc.dram_tensor("out", [len(rsched), BLK, (max(rsched) // BLK) * P],
                                out_dt, kind="ExternalOutput").ap()
    else:
        x_ap = nc.dram_tensor("x", [C, R_PER_CORE], x_dt, kind="ExternalInput").ap()
        out_ap = nc.dram_tensor("out", [R_PER_CORE, P], out_dt,
                                kind="ExternalOutput").ap()
    maug_ap = nc.dram_tensor("maug", [KCH, BLK, NA], F32, kind="ExternalInput").ap()
    NS = len(rsched)
    # live slice-level PSUM tiles (ring slots sized by the largest slice)
    # must fit in the 8 PSUM banks
    u_bufs = max(2, ln_lag + 1)
    assert u_bufs * max(rsched) <= 1024

    with tile.TileContext(nc) as tc, ExitStack() as ctx:
        const_pool = ctx.enter_context(tc.tile_pool(name="const", bufs=1))
        x_pool = ctx.enter_context(tc.tile_pool(name="xin", bufs=NS))
        e_pool = ctx.enter_context(tc.tile_pool(name="e", bufs=e_bufs))
        lnu_pool = ctx.enter_context(tc.tile_pool(name="lnu", bufs=lnu_bufs))
        o_pool = ctx.enter_context(tc.tile_pool(name="o", bufs=o_bufs))
        s_pool = ctx.enter_context(tc.tile_pool(name="small", bufs=4))
        u_pool = ctx.enter_context(tc.tile_pool(name="u", bufs=u_bufs, space="PSUM"))

        maug_sb = const_pool.tile([BLK, KCH * NA], BF16)
        nc.gpsimd.dma_start(
            maug_sb[:].rearrange("p (k n) -> p k n", n=NA),
            maug_ap[:, :, :].rearrange("k p n -> p k n"),
        )
        zwarm = None
        if pe_warm:
            zwarm = const_pool.tile([BLK, 2 * BLK], BF16)
            nc.vector.memset(zwarm[:], 0.0)

        loop_cm = (
            tc.For_i(0, abs(bench_iters), 1) if bench_iters else nullcontext()
        )
        ctx.enter_context(loop_cm)

        if pe_warm and not dma_only:
            # dummy matmuls keep the PE p-state ramp going during the DMA
            # fill window so the first real matmuls run at full clock
            # (borrows a u-pool slot; freed before slice 1 needs it)
            wu = u_pool.tile([BLK, (max(rsched) // BLK) * UST], F32, tag="u")
            for i in range(pe_warm):
                nc.tensor.matmul(
                    wu[:, 0:2 * BLK], zwarm[:, 0:BLK], zwarm[:, 0:2 * BLK],
                    start=True, stop=True,
                )

        x_t3 = None if packed_io else x_ap.rearrange("(k p) r -> k p r", p=BLK)
        RSMAX = max(rsched)
        kper = KCH // in_split

        # phase A: all input DMAs, one (or in_split) per slice, on SP HWDGE
        xs_tiles = []
        r0 = 0
        for si, rs_ in enumerate(rsched):
            xs = x_pool.tile([BLK, KCH * RSMAX], x_dt, tag="xs")
            if compute_only:
                nc.vector.memset(xs[:, 0:8], 0.0)
            elif packed_io:
                isp = fill_split if (si == 0 and fill_split > 1) else in_split
                step = KCH * rs_ // isp
                for d in range(isp):
                    nc.sync.dma_start(
                        xs[:, d * step:(d + 1) * step],
                        x_ap[si, :, d * step:(d + 1) * step],
                    )
            else:
                for d in range(in_split):
                    nc.sync.dma_start(
                        xs[:, d * kper * rs_:(d + 1) * kper * rs_].rearrange(
                            "p (k r) -> p k r", r=rs_
                        ),
                        x_t3[d * kper:(d + 1) * kper, :, r0:r0 + rs_].rearrange(
                            "k p r -> p k r"
                        ),
                    )
            xs_tiles.append(xs)
            r0 += rs_

        # phase B: per slice: exp -> matmuls; Ln/subtract/out trail ln_lag
        # slices behind so the in-order ACT stream never waits on the PE.
        NAW = NA  # columns per block read by the batched Ln ([numer | s])

        def emit_out_dma(BPRS, si_, r0_, outs):
            ob = out_blocks if out_blocks else BPRS
            for b0 in range(0, BPRS, ob):
                nb = min(ob, BPRS - b0)
                if packed_io:
                    getattr(nc, out_eng).dma_start(
                        out_ap[si_, :, b0 * P:(b0 + nb) * P],
                        outs[:, b0 * P:(b0 + nb) * P],
                    )
                else:
                    getattr(nc, out_eng).dma_start(
                        out_ap[r0_ + b0 * BLK:r0_ + (b0 + nb) * BLK, :].rearrange(
                            "(b p) c -> p b c", p=BLK
                        ),
                        outs[:, b0 * P:(b0 + nb) * P].rearrange(
                            "p (b c) -> p b c", c=P
                        ),
                    )

        def emit_ln(rs_, u, r0_, si_, skip_dma=False, per_block=False):
            if stage in ("exp", "mm", "mm1"):
                return
            BPRS = rs_ // BLK
            outs = o_pool.tile([BLK, (RSMAX // BLK) * P], out_dt, tag="outs")
            u3 = u[:, 0:BPRS * UST].rearrange("p (b x) -> p b x", x=UST)
            lnu = lnu_pool.tile([BLK, (RSMAX // BLK) * NAW], F32, tag="lnu")
            lnu3 = lnu[:, 0:BPRS * NAW].rearrange("p (b x) -> p b x", x=NAW)
            if not per_block:
                # one ACT op: ln of numer AND s for every block of the slice
                nc.scalar.activation(
                    lnu3[:, :, :], u3[:, :, 0:NAW],
                    mybir.ActivationFunctionType.Ln,
                )
            if stage == "ln" and not per_block:
                return
            for b in range(BPRS):
                if per_block:
                    # drain mode: Ln -> subtract -> out-DMA pipelined per
                    # block so the kernel tail is short
                    nc.scalar.activation(
                        lnu3[:, b:b + 1, :], u3[:, b:b + 1, 0:NAW],
                        mybir.ActivationFunctionType.Ln,
                    )
                seng = nc.vector if (sub_eng == "vector" or b % 2 == 0) else nc.gpsimd
                seng.tensor_scalar(
                    outs[:, b * P:(b + 1) * P],
                    lnu[:, b * NAW:b * NAW + P],
                    lnu[:, b * NAW + P:b * NAW + P + 1],
                    None,
                    op0=mybir.AluOpType.subtract,
                )
                if per_block and not skip_dma:
                    if packed_io:
                        getattr(nc, out_eng).dma_start(
                            out_ap[si_, :, b * P:(b + 1) * P],
                            outs[:, b * P:(b + 1) * P],
                        )
                    else:
                        getattr(nc, out_eng).dma_start(
                            out_ap[r0_ + b * BLK:r0_ + (b + 1) * BLK, :].rearrange(
                                "(o p) c -> p o c", p=BLK
                            ),
                            outs[:, b * P:(b + 1) * P].rearrange(
                                "p (o c) -> p o c", c=P
                            ),
                        )
            if skip_dma or per_block:
                return
            emit_out_dma(BPRS, si_, r0_, outs)

        if dma_only:
            r0 = 0
            for si, rs_ in enumerate(rsched):
                BPRS = rs_ // BLK
                outs = o_pool.tile([BLK, (RSMAX // BLK) * P], out_dt, tag="outs")
                nc.vector.memset(outs[:, 0:8], 0.0)
                emit_out_dma(BPRS, si, r0, outs)
                r0 += rs_
        else:
            pending = []
            r0 = 0
            for si, rs_ in enumerate(rsched):
                BPRS = rs_ // BLK
                xs = xs_tiles[si]
                e = e_pool.tile([BLK, KCH * RSMAX], BF16, tag="e")
                esp = fill_split if (si == 0 and fill_split > 1 and packed_io
                                     and not compute_only) else 1
                if exp_rsplit > 1 and esp == 1:
                    # split the slice's exp by ROW halves (strided 3D APs):
                    # the first blocks' matmuls start after half the exp
                    x3 = xs[:, 0:KCH * rs_].rearrange("p (k r) -> p k r", r=rs_)
                    e3 = e[:, 0:KCH * rs_].rearrange("p (k r) -> p k r", r=rs_)
                    rstep = rs_ // exp_rsplit
                    for h in range(exp_rsplit):
                        nc.scalar.activation(
                            e3[:, :, h * rstep:(h + 1) * rstep],
                            x3[:, :, h * rstep:(h + 1) * rstep],
                            mybir.ActivationFunctionType.Exp,
                        )
                else:
                    estep = KCH * rs_ // esp
                    for d in range(esp):
                        nc.scalar.activation(
                            e[:, d * estep:(d + 1) * estep],
                            xs[:, d * estep:(d + 1) * estep],
                            mybir.ActivationFunctionType.Exp,
                        )
                u = u_pool.tile([BLK, (max(rsched) // BLK) * UST], F32, tag="u")
                if stage != "exp":
                    kch_eff = 1 if stage == "mm1" else KCH
                    for b in range(BPRS):
                        for k in range(kch_eff):
                            nc.tensor.matmul(
                                u[:, b * UST:b * UST + NA],
                                e[:, k * rs_ + b * BLK:k * rs_ + (b + 1) * BLK],
                                maug_sb[:, k * NA:(k + 1) * NA],
                                start=(k == 0),
                                stop=(k == kch_eff - 1),
                            )
                pending.append((rs_, u, r0, si))
                if len(pending) > ln_lag:
                    emit_ln(*pending.pop(0), skip_dma=compute_only)
                r0 += rs_
            for li, args in enumerate(pending):
                emit_ln(*args, skip_dma=compute_only,
                        per_block=tail_split and li == len(pending) - 1)
    try:
        nc.compile()
    finally:
        _restore_tabs()
    return nc


def make_maug2(alloW, phone_arc_labels, phoneme_arc_labels):
    """[M + (1-mrow)/P | ones] for build_graph_v2 (redistribution folded)."""
    alloW = np.asarray(alloW, dtype=np.float64).reshape(-1)
    phone = np.asarray(phone_arc_labels).astype(np.int64).reshape(-1)
    phoneme = np.asarray(phoneme_arc_labels).astype(np.int64).reshape(-1)
    M = np.zeros((C, P), dtype=np.float64)
    np.add.at(M, (phone, phoneme), np.exp(alloW))
    NA = P + 1
    maug = np.empty((C, NA), dtype=np.float64)
    maug[:, :P] = M + (1.0 - M.sum(axis=1, keepdims=True)) / P
    maug[:, P] = 1.0
    return maug.astype(np.float32).reshape(KCH, BLK, NA)


def build_empty_loop(bench_iters, n_pools=8):
    """For_i loop with a near-empty body: measures the per-iteration
    back-edge/reset overhead that every slope measurement includes."""
    from contextlib import ExitStack

    nc = bacc.Bacc("TRN2", target_bir_lowering=False, debug=False, num_devices=1)
    out_ap = nc.dram_tensor("out", [BLK, 8], F32, kind="ExternalOutput").ap()
    with tile.TileContext(nc) as tc, ExitStack() as ctx:
        pools = [ctx.enter_context(tc.tile_pool(name=f"p{i}", bufs=3))
                 for i in range(n_pools)]
        with tc.For_i(0, bench_iters, 1):
            t = pools[0].tile([BLK, 8], F32, tag="t")
            nc.vector.memset(t[:], 1.0)
            nc.sync.dma_start(out_ap[:, :], t[:])
    nc.compile()
    return nc


def build_loopcount_probe(bench_iters):
    """Tiny graph whose output literally counts loop iterations: each
    iteration DMA-accumulates a tile of ones into the output."""
    from contextlib import ExitStack

    nc = bacc.Bacc("TRN2", target_bir_lowering=False, debug=False, num_devices=1)
    out_ap = nc.dram_tensor("out", [BLK, BLK], F32, kind="ExternalOutput").ap()
    with tile.TileContext(nc) as tc, ExitStack() as ctx:
        pool = ctx.enter_context(tc.tile_pool(name="ones", bufs=1))
        ones = pool.tile([BLK, BLK], F32)
        nc.vector.memset(ones[:], 1.0)
        with tc.For_i(0, bench_iters, 1):
            nc.gpsimd.dma_start(out_ap[:, :], ones[:],
                                accum_op=mybir.AluOpType.add)
    nc.compile()
    return nc


def make_maug(alloW, phone_arc_labels, phoneme_arc_labels):
    alloW = np.asarray(alloW, dtype=np.float64).reshape(-1)
    phone = np.asarray(phone_arc_labels).astype(np.int64).reshape(-1)
    phoneme = np.asarray(phoneme_arc_labels).astype(np.int64).reshape(-1)
    M = np.zeros((C, P), dtype=np.float64)
    np.add.at(M, (phone, phoneme), np.exp(alloW))
    maug = np.empty((C, NAUG), dtype=np.float64)
    maug[:, :P] = M
    maug[:, P] = M.sum(axis=1) / P          # U[:,256] = w/P
    maug[:, P + 1] = 1.0                    # U[:,257] = s  (softmax denom)
    maug[:, P + 2] = 1.0 / P                # U[:,258] = s/P
    return maug.astype(np.float32).reshape(KCH, BLK, NAUG)


def pack_x_core(x_t, rsched, np_dtype):
    """[C, R_PER_CORE] -> [NS, BLK, KCH*rsmax] (each slice contiguous per
    partition: partition p holds rows {k*128+p} k-major; short slices are
    left-justified in their padded row)."""
    rsmax = max(rsched)
    out = np.zeros((len(rsched), BLK, KCH * rsmax), dtype=np_dtype)
    r0 = 0
    for si, rs_ in enumerate(rsched):
        sl = x_t[:, r0:r0 + rs_].reshape(KCH, BLK, rs_).transpose(1, 0, 2)
        out[si, :, 0:KCH * rs_] = sl.reshape(BLK, KCH * rs_)
        r0 += rs_
    return out


def unpack_out_core(arr, rsched):
    """[NS, BLK, (rsmax//BLK)*P] device layout -> [R_PER_CORE, P] row-major."""
    rsmax = max(rsched)
    a = np.asarray(arr, dtype=np.float32).reshape(
        len(rsched), BLK, rsmax // BLK, P)
    parts = []
    for si, rs_ in enumerate(rsched):
        parts.append(a[si, :, 0:rs_ // BLK, :].transpose(1, 0, 2).reshape(rs_, P))
    return np.concatenate(parts, axis=0)


def make_in_maps(hs_rows_by_cols, maug, x_bf16=False):
    """Shard [ROWS, C] f32 over cores; each shard shipped transposed [C, r]."""
    xdt = ml_dtypes.bfloat16 if x_bf16 else np.float32
    return [
        {
            "x": np.ascontiguousarray(
                hs_rows_by_cols[i * R_PER_CORE:(i + 1) * R_PER_CORE].T
            ).astype(xdt),
            "maug": maug,
        }
        for i in range(N_CORES)
    ]


_NC = None


def build_best(**kw):
    fn = {"t": build_graph_t, "v2": build_graph_v2}[BEST_BUILDER]
    return fn(**BEST_CFG, **kw)


def _get_nc():
    global _NC
    if _NC is None:
        _NC = build_best()
    return _NC


def best_rsched():
    return BEST_CFG.get("rsched") or [BEST_CFG.get("rs", 512)] * (
        R_PER_CORE // BEST_CFG.get("rs", 512))


def best_x_np_dtype():
    if BEST_CFG.get("x_fp8"):
        return ml_dtypes.float8_e4m3
    if BEST_CFG.get("x_bf16", BEST_BUILDER == "v2"):
        return ml_dtypes.bfloat16
    return np.float32


def make_best_in_maps(hs_rows_by_cols, alloW, phone_arc_labels, phoneme_arc_labels):
    """Per-core input dicts for the shipped config (shard, transpose to
    [C, r], pack slices, downcast)."""
    if BEST_BUILDER == "v2":
        maug = make_maug2(alloW, phone_arc_labels, phoneme_arc_labels)
    else:
        maug = make_maug(alloW, phone_arc_labels, phoneme_arc_labels)
    xdt = best_x_np_dtype()
    in_maps = []
    for i in range(N_CORES):
        xt = np.ascontiguousarray(
            hs_rows_by_cols[i * R_PER_CORE:(i + 1) * R_PER_CORE].T)
        if BEST_CFG.get("packed_io"):
            x = pack_x_core(xt, best_rsched(), xdt)
        else:
            x = xt.astype(xdt)
        in_maps.append({"x": x, "maug": maug})
    return in_maps


def run(hs_pad, alloW, phone_arc_labels, phoneme_arc_labels, n_phonemes, trace=False):
    import time

    assert int(n_phonemes) == P
    hs = np.ascontiguousarray(np.asarray(hs_pad, dtype=np.float32)).reshape(ROWS, C)
    # data-parallel shard over rows; shards handed to the device pre-transposed
    # ([C, r] layout) so the contraction dim lands on SBUF partitions with no
    # on-chip transpose
    in_maps = make_best_in_maps(hs, alloW, phone_arc_labels, phoneme_arc_labels)
    nc = _get_nc()
    last_err = None
    for attempt in range(7):
        try:
            res = bass_utils.run_bass_kernel_spmd(
                nc, in_maps, core_ids=list(range(N_CORES)), trace=trace
            )
            break
        except Exception as e:  # transient NRT exec-unit errors recover on retry
            last_err = e
            time.sleep(min(2.0 * (attempt + 1), 10.0))
            if attempt >= 1:
                # an exec-unit-unrecoverable error wedges the PJRT client;
                # a backend reset (fresh executables) recovers where a
                # plain retry would keep failing
                try:
                    import jax
                    jax.clear_backends()
                except Exception:
                    pass
    else:
        raise last_err
    outs = []
    for i in range(N_CORES):
        o = res.results[i]["out"]
        if BEST_CFG.get("packed_io"):
            o = unpack_out_core(o, best_rsched())
        outs.append(np.asarray(o, dtype=np.float32).reshape(R_PER_CORE, P))
    return np.concatenate(outs, axis=0).reshape(B, T, P), res


def kernel(hs_pad, alloW, phone_arc_labels, phoneme_arc_labels, n_phonemes):
    out, _ = run(hs_pad, alloW, phone_arc_labels, phoneme_arc_labels, n_phonemes)
    return out
